# revision 1
# baseline (speedup 1.0000x reference)
"""BertSelfAttention on 8 Trainium2 NeuronCores.

Problem: B=2, S=2048, H=1024, 16 heads x 64. Sharding: batch x head-group
(2 batches x 4 head-groups of 4 heads = 8 cores). Each core computes
q/k/v projections for its 4 heads and full attention over them.

Per-core device kernel (SPMD, one program; matmul operands in fp16,
accumulation fp32):
  inputs (host-prepared):
    xT    [1024, 2048]  x[b].T, fp16
    wqT/wkT/wvT [1024, 256]  W.T columns for this head group, fp16
    bqk   [128, 4]      q/k biases per o-chunk (per-partition layout)
    bvb   [128, 260]    v bias + ones column, broadcast across partitions
    mb    [128, 16]     additive mask bias per key position ((1-m)*-1e30)
  output:
    out   [2048, 256]   attention output, natural [s, head-local o]

  phase 1: projections (contract over feature dim)
    qT,kT [256, 2048] (head dim on partitions), v [2048, 4x(64+1)]
    (65th column of each head block = 1.0 -> the PV matmul also computes
     the softmax denominator). sc-outer accumulation passes so each psum
    chunk retires right after its 8-matmul chain.
  phase 2: per head pair (row-tiled K=64 matmuls, partitions e*64..):
    scoresT[k,q] = kT.T @ qT ; probs = exp(scores/8 + maskbias) on ACT
    (no max subtraction: scores are ~N(0,1), exp range is safe)
    pv[65, q]    = sum_k V'[k,65].T @ probs[k,q]   (row 64 = sum exp)
    PE-transpose pv -> [q, 65], out[q,d] = pv[q,d] * 1/pv[q,64]
"""

import sys

sys.path.insert(0, "/opt/trn_rl_repo")

import numpy as np

import concourse.bass as bass
import concourse.tile as tile
from concourse.masks import make_identity
from concourse import bacc, mybir
from concourse.bass_utils import run_bass_kernel_spmd

F32 = mybir.dt.float32
F16 = mybir.dt.float16
EXP = mybir.ActivationFunctionType.Exp

B, S, H = 2, 2048, 1024
NH, HD = 16, 64
G = 4                 # head-groups (cores per batch)
NHL = NH // G         # heads per core
O = NHL * HD          # 256 output features per core
IC = H // 128         # 8 contraction chunks
KC = S // 128         # 16 key chunks
QCHUNK = 1024         # q processed in chunks of 1024
NQ = S // QCHUNK
NEG = -1.0e30


def build_nc():
    nc = bacc.Bacc(None, target_bir_lowering=False)
    xT = nc.declare_dram_parameter("xT", [H, S], F16, isOutput=False)
    wqT = nc.declare_dram_parameter("wqT", [H, O], F16, isOutput=False)
    wkT = nc.declare_dram_parameter("wkT", [H, O], F16, isOutput=False)
    wvT = nc.declare_dram_parameter("wvT", [H, O], F16, isOutput=False)
    bqk = nc.declare_dram_parameter("bqk", [128, 4], F32, isOutput=False)
    bvb = nc.declare_dram_parameter("bvb", [128, NHL * (HD + 1)], F16,
                                    isOutput=False)
    mb = nc.declare_dram_parameter("mb", [128, KC], F32, isOutput=False)
    out = nc.declare_dram_parameter("out", [S, O], F32, isOutput=True)

    with tile.TileContext(nc) as tc:
        with tc.tile_pool(name="consts", bufs=1) as consts, \
             tc.tile_pool(name="persist", bufs=1) as persist:
            ident = consts.tile([128, 128], F32, tag="ident")
            make_identity(nc, ident)
            mb_sb = consts.tile([128, KC], F32, tag="mb")
            bqk_sb = consts.tile([128, 4], F32, tag="bqk")
            bvb_sb = consts.tile([128, NHL * (HD + 1)], F16, tag="bvb")

            # persistent activations
            qT = [persist.tile([128, S], F16, tag=f"qT{i}", name=f"qT{i}") for i in range(2)]
            kT = [persist.tile([128, S], F16, tag=f"kT{i}", name=f"kT{i}") for i in range(2)]
            vS = [persist.tile([128, NHL * (HD + 1)], F16, tag=f"v{i}", name=f"v{i}")
                  for i in range(KC)]

            # ---------------- phase 1: projections ----------------
            # sc-outer accumulation passes: each 512-chunk retires (psum ->
            # sbuf copy) right after its 8-matmul chain, so the last copy
            # trails the last matmul by <1us and the PE never idles long
            # enough to re-throttle at the proj->attention boundary.
            with tc.tile_pool(name="xt", bufs=1) as xtp, \
                 tc.tile_pool(name="w", bufs=1) as wp, \
                 tc.tile_pool(name="pps", bufs=6, space="PSUM") as pps:
                xt = [xtp.tile([128, S], F16, tag=f"xt{i}", name=f"xt{i}")
                      for i in range(IC)]
                wq = [wp.tile([128, O], F16, tag=f"wq{i}", name=f"wq{i}")
                      for i in range(IC)]
                wk = [wp.tile([128, O], F16, tag=f"wk{i}", name=f"wk{i}")
                      for i in range(IC)]
                wv = [wp.tile([128, O], F16, tag=f"wv{i}", name=f"wv{i}")
                      for i in range(IC)]
                # x + q/k weights interleaved so the first matmuls can
                # start as soon as chunk 0 lands; v weights later
                for i in range(IC):
                    nc.sync.dma_start(out=xt[i], in_=xT[i * 128:(i + 1) * 128, :])
                    nc.sync.dma_start(out=wq[i], in_=wqT[i * 128:(i + 1) * 128, :])
                    nc.sync.dma_start(out=wk[i], in_=wkT[i * 128:(i + 1) * 128, :])
                # const DMAs + ACT exp-table warmup issue after the
                # x/weight triggers the first matmuls are waiting on
                nc.sync.dma_start(out=mb_sb, in_=mb[:, :])
                nc.sync.dma_start(out=bqk_sb, in_=bqk[:, :])
                nc.sync.dma_start(out=bvb_sb, in_=bvb[:, :])
                dummy = consts.tile([128, 1], F32, tag="dummy")
                nc.vector.memset(dummy, 0.0)
                nc.scalar.activation(dummy, dummy, EXP)
                for i in range(IC):
                    nc.sync.dma_start(out=wv[i], in_=wvT[i * 128:(i + 1) * 128, :])

                def qk_pass(wt, ot, dest, bcol):
                    for sc in range(4):
                        ps = pps.tile([128, 512], F32, tag="pp",
                                      name=f"pp{bcol}_{sc}")
                        for i in range(IC):
                            nc.tensor.matmul(
                                ps,
                                lhsT=wt[i][:, ot * 128:(ot + 1) * 128],
                                rhs=xt[i][:, sc * 512:(sc + 1) * 512],
                                start=(i == 0), stop=(i == IC - 1))
                        nc.vector.tensor_scalar_add(
                            dest[:, sc * 512:(sc + 1) * 512], ps,
                            bqk_sb[:, bcol:bcol + 1])

                def v_pass():
                    bvview = bvb_sb.rearrange("p (h d) -> p h d", h=NHL)
                    for sc in range(KC):
                        ps = pps.tile([128, O], F32, tag="pp", name=f"ppv{sc}")
                        for i in range(IC):
                            nc.tensor.matmul(
                                ps,
                                lhsT=xt[i][:, sc * 128:(sc + 1) * 128],
                                rhs=wv[i],
                                start=(i == 0), stop=(i == IC - 1))
                        vview = vS[sc].rearrange("p (h d) -> p h d", h=NHL)
                        nc.vector.tensor_add(
                            vview[:, :, 0:HD],
                            ps.rearrange("p (h d) -> p h d", h=NHL),
                            bvview[:, :, 0:HD])
                        nc.vector.tensor_copy(
                            vview[:, :, HD:HD + 1], bvview[:, :, HD:HD + 1])

                qk_pass(wq, 0, qT[0], 0)
                qk_pass(wk, 0, kT[0], 2)
                v_pass()
                qk_pass(wq, 1, qT[1], 1)
                qk_pass(wk, 1, kT[1], 3)

            # ---------------- phase 2: attention (paired heads) ----------
            with tc.tile_pool(name="scps", bufs=1, space="PSUM") as scps, \
                 tc.tile_pool(name="pvps", bufs=1, space="PSUM") as pvps, \
                 tc.tile_pool(name="pbp", bufs=3) as pbp, \
                 tc.tile_pool(name="tailp", bufs=2) as tailp:
                for hp in range(2):          # head pair = o-chunk
                    for qc in range(NQ):
                        pv = [pvps.tile([HD + 1, QCHUNK], F32, tag=f"pv{e}",
                                        name=f"pv{e}") for e in range(2)]
                        for kc in range(KC):
                            sc_ps = [scps.tile([128, QCHUNK], F32,
                                               tag=f"sc{e}", name=f"sc{e}")
                                     for e in range(2)]
                            # scoresT: row-tiled pair (partitions e*64..)
                            for n in range(QCHUNK // 512):
                                for e in range(2):
                                    lo = e * 64
                                    nc.tensor.matmul(
                                        sc_ps[e][:, n * 512:(n + 1) * 512],
                                        lhsT=kT[hp][lo:lo + 64,
                                                    kc * 128:(kc + 1) * 128],
                                        rhs=qT[hp][lo:lo + 64,
                                                   qc * QCHUNK + n * 512:
                                                   qc * QCHUNK + (n + 1) * 512],
                                        start=True, stop=True)
                            for e in range(2):
                                pb = pbp.tile([128, QCHUNK], F16,
                                              tag=f"pb{e}", name=f"pb{e}")
                                nc.scalar.activation(
                                    pb, sc_ps[e], EXP,
                                    bias=mb_sb[:, kc:kc + 1], scale=0.125)
                                hh = 2 * hp + e
                                for n in range(QCHUNK // 512):
                                    nc.tensor.matmul(
                                        pv[e][:, n * 512:(n + 1) * 512],
                                        lhsT=vS[kc][:, hh * 65:hh * 65 + 65],
                                        rhs=pb[:, n * 512:(n + 1) * 512],
                                        start=(kc == 0), stop=(kc == KC - 1))
                        # tail: normalize + transpose to [q, d] and store
                        for e in range(2):
                            hh = 2 * hp + e
                            ovt = tailp.tile([HD + 1, QCHUNK], F32,
                                             tag=f"ovt{e}", name=f"ovt{e}")
                            nc.vector.tensor_copy(ovt, pv[e])
                            nj = QCHUNK // 128
                            # 128-padded blocks keep each transpose in one
                            # PSUM bank
                            tr = pvps.tile([128, nj, 128], F32, tag=f"pv{e}",
                                           name=f"tr{e}")
                            for jb in range(nj):
                                nc.tensor.transpose(
                                    tr[:, jb, 0:HD + 1],
                                    ovt[:, jb * 128:(jb + 1) * 128],
                                    ident[0:HD + 1, 0:HD + 1])
                            rc = tailp.tile([128, nj], F32, tag=f"rc{e}",
                                            name=f"rc{e}")
                            nc.vector.reciprocal(rc, tr[:, :, 64])
                            osb = tailp.tile([128, nj * HD], F32,
                                             tag=f"osb{e}", name=f"osb{e}")
                            for jb in range(nj):
                                nc.vector.tensor_scalar_mul(
                                    osb[:, jb * HD:(jb + 1) * HD],
                                    tr[:, jb, 0:HD], rc[:, jb:jb + 1])
                            dst = out[qc * QCHUNK:(qc + 1) * QCHUNK,
                                      hh * HD:(hh + 1) * HD]
                            dst = dst.rearrange("(j p) d -> p j d", p=128)
                            nc.sync.dma_start(
                                out=dst,
                                in_=osb.rearrange("p (j d) -> p j d", j=nj))
    nc.finalize()
    return nc


_NC_CACHE = None


def _get_nc():
    global _NC_CACHE
    if _NC_CACHE is None:
        _NC_CACHE = build_nc()
    return _NC_CACHE


def make_in_maps(inputs, attention_mask, Wq, bq, Wk, bk, Wv, bv):
    x = np.asarray(inputs, dtype=np.float32)
    mask = np.asarray(attention_mask)
    Wq = np.asarray(Wq, dtype=np.float32)
    Wk = np.asarray(Wk, dtype=np.float32)
    Wv = np.asarray(Wv, dtype=np.float32)
    bq = np.asarray(bq, dtype=np.float32)
    bk = np.asarray(bk, dtype=np.float32)
    bv = np.asarray(bv, dtype=np.float32)

    xTb = [np.ascontiguousarray(x[b].T).astype(np.float16) for b in range(B)]
    mbb = [np.ascontiguousarray(
        ((1.0 - mask[b].astype(np.float32)) * NEG).reshape(KC, 128).T)
        for b in range(B)]
    in_maps = []
    for c in range(8):
        b, g = c // G, c % G
        cols = slice(g * O, (g + 1) * O)
        bqs, bks = bq[cols], bk[cols]
        bvc = np.concatenate(
            [np.concatenate([bv[cols][h * 64:(h + 1) * 64], [1.0]])
             for h in range(NHL)]).astype(np.float32)
        bvbc = np.ascontiguousarray(np.broadcast_to(bvc[None, :], (128, len(bvc))))
        in_maps.append({
            "xT": xTb[b],
            "wqT": np.ascontiguousarray(Wq.T[:, cols]).astype(np.float16),
            "wkT": np.ascontiguousarray(Wk.T[:, cols]).astype(np.float16),
            "wvT": np.ascontiguousarray(Wv.T[:, cols]).astype(np.float16),
            "bqk": np.ascontiguousarray(
                np.stack([bqs[:128], bqs[128:], bks[:128], bks[128:]], axis=1)),
            "bvb": bvbc.astype(np.float16),
            "mb": mbb[b],
        })
    return in_maps


def assemble(results):
    outs = [results[c]["out"] for c in range(8)]
    full = np.stack(
        [np.concatenate(outs[b * G:(b + 1) * G], axis=1) for b in range(B)])
    return np.ascontiguousarray(full.astype(np.float32))


def kernel(**inputs) -> np.ndarray:
    nc = _get_nc()
    in_maps = make_in_maps(**inputs)
    res = run_bass_kernel_spmd(nc, in_maps, core_ids=list(range(8)))
    return assemble(res.results)

